# revision 2
# baseline (speedup 1.0000x reference)
"""XL-BOMD rank-4 Krylov propagation (EnergyXL) on 8 TRN2 NeuronCores.

Monomial-Krylov reformulation: the reference's Gram-Schmidt basis is
mathematically irrelevant to the output because
    out = -V (W^T W)^{-1} W^T dDS
is invariant under any change of Krylov basis V (W = L V transforms
covariantly).  So per molecule we compute the raw sandwich chain
    m_0 = D - P,  m_{t+1} = R m_t R        (8 PE matmuls, bf16)
its 9 Hankel moments  s_t = <m_a, m_b> (a+b = t)  via the identity
<R a R, b> = <a, R b R>, solve the 4x4 Hankel system
    A[a,c] = s[a+c+2] - 2 s[a+c+1] + s[a+c],  rhs[a] = s[a+1] - s[a]
(batched no-pivot Gaussian elimination over mol-partition layout), and
emit  out = -sum_a y_a m_a.   Rel-err vs fp64 reference: ~2.8e-3
(bf16 matmul chain + bf16 product tiles, fp32 moment accumulation).

Layout: molecules processed in PAIRS sharing [128, 384] (hi rows 0:128)
and [64, 384] (lo rows 128:192) tiles, mol A cols 0:192 / mol B cols
192:384.  Moments: DVE/Pool elementwise products -> PE ones-matmul
column sums (product tiles are symmetric, so colsum == rowsum) into a
per-pair [9, 384] PSUM bank -> one DVE reduce -> [9, 2] per pair.
Per group of 32 mols: one 32x32 DVE transpose + one batched solve.
"""

import sys

sys.path.insert(0, "/opt/trn_rl_repo")

import numpy as np

import concourse.bass as bass
import concourse.bacc as bacc
import concourse.tile as tile
from concourse import mybir
from concourse.bass_utils import run_bass_kernel_spmd

F32 = mybir.dt.float32
BF16 = mybir.dt.bfloat16
ALU = mybir.AluOpType
ACTF = mybir.ActivationFunctionType

NMOL, N, RANK = 512, 192, 4
NCORES = 8
MPC = NMOL // NCORES      # 64 molecules per core
HI, LO = 128, 64
GRP = 32                  # molecules per solve group
SUB = 8                   # molecules per DMA batch

# moment t -> (a, b) with a+b = t over stored m_0..m_4
MOM_PAIRS = [(0, 0), (0, 1), (1, 1), (1, 2), (2, 2), (2, 3), (3, 3), (3, 4), (4, 4)]

# --- engine assignment tables (tuned against TimelineSim) ---
# product index 0..7 = q1,m1,q2,m2,q3,m3,q4,m4 ; m4 (idx 7) is not copied.
CP_HI = ["act", "act", "act", "act", "act", "act", "act", "act"]
CP_LO = ["dve", "dve", "dve", "dve", "dve", "dve", "dve", "dve"]
MOM_ENG = ["dve"] * 9
CMB_HI = ["act", "dve", "dve", "dve"]   # per a=0..3 (a=0 may be "act")
CMB_LO = ["act", "dve", "dve", "dve"]


def _eng(nc, name):
    return {"dve": nc.vector, "pool": nc.gpsimd}[name]


def build_core_kernel(n_mols=MPC):
    nc = bacc.Bacc(None, target_bir_lowering=False, enable_partition_id=False)
    D = nc.dram_tensor("D", [n_mols, N, N], F32, kind="ExternalInput")
    P = nc.dram_tensor("P", [n_mols, N, N], F32, kind="ExternalInput")
    R = nc.dram_tensor("Rb", [n_mols, N, N], BF16, kind="ExternalInput")
    OUT = nc.dram_tensor("OUT", [n_mols, N, N], F32, kind="ExternalOutput")
    with tile.TileContext(nc) as tc:
        _body(nc, tc, D, P, R, OUT, n_mols)
    nc.finalize()
    return nc


def _body(nc, tc, D, P, R, OUT, n_mols):
    import contextlib

    ctx = contextlib.ExitStack()
    with ctx:
        consts = ctx.enter_context(tc.tile_pool(name="consts", bufs=1))
        stg = ctx.enter_context(tc.tile_pool(name="stg", bufs=2))
        mpool = ctx.enter_context(tc.tile_pool(name="mpool", bufs=1))
        qpool = ctx.enter_context(tc.tile_pool(name="qpool", bufs=2))
        scrp = ctx.enter_context(tc.tile_pool(name="scrp", bufs=2))
        outp = ctx.enter_context(tc.tile_pool(name="outp", bufs=2))
        smallp = ctx.enter_context(tc.tile_pool(name="smallp", bufs=2))
        solvep = ctx.enter_context(tc.tile_pool(name="solvep", bufs=2))
        ps_ch = ctx.enter_context(tc.tile_pool(name="ps_ch", bufs=2, space="PSUM"))
        ps_sm = ctx.enter_context(tc.tile_pool(name="ps_sm", bufs=2, space="PSUM"))
        ps_bc = ctx.enter_context(tc.tile_pool(name="ps_bc", bufs=2, space="PSUM"))

        nmom = len(MOM_PAIRS)
        sel_hi = consts.tile([HI, nmom * nmom], BF16)
        nc.vector.memset(sel_hi, 0.0)
        sel_lo = consts.tile([LO, nmom * nmom], BF16)
        nc.vector.memset(sel_lo, 0.0)
        for t in range(nmom):
            nc.vector.memset(sel_hi[:, nmom * t + t: nmom * t + t + 1], 1.0)
            nc.vector.memset(sel_lo[:, nmom * t + t: nmom * t + t + 1], 1.0)
        ones32 = consts.tile([GRP, HI], F32)
        nc.vector.memset(ones32, 1.0)
        id32 = consts.tile([GRP, GRP], F32)
        idt = consts.tile([GRP, GRP], mybir.dt.int32)
        nc.gpsimd.iota(idt, pattern=[[-1, GRP]], base=0, channel_multiplier=1)
        nc.vector.tensor_scalar(out=id32, in0=idt, scalar1=0, scalar2=None,
                                op0=ALU.is_equal)

        for g in range(n_mols // GRP):
            _group(nc, tc, D, P, R, OUT, g, consts, stg, mpool, qpool, scrp,
                   outp, smallp, solvep, ps_ch, ps_sm, ps_bc,
                   sel_hi, sel_lo, ones32, id32)


def _group(nc, tc, D, P, R, OUT, g, consts, stg, mpool, qpool, scrp, outp,
           smallp, solvep, ps_ch, ps_sm, ps_bc, sel_hi, sel_lo, ones32, id32):
    m0g = g * GRP
    npairs = GRP // 2
    nsub = GRP // SUB

    SG = solvep.tile([GRP, GRP], F32, tag="SG")

    # --- per-pair: staged DMA (rotating per 8-mol sub-batch), dds, chain,
    # moments ---
    pair_ms = []   # per pair: list of 4 (hi, lo) m tiles (bf16, [*, 384])
    cur_stage = None
    for k in range(npairs):
        s, i = divmod(2 * k, SUB)
        if i == 0:
            ms = m0g + s * SUB
            dh = stg.tile([HI, SUB, N], F32, tag="dh")
            dl = stg.tile([LO, SUB, N], F32, tag="dl")
            ph = stg.tile([HI, SUB, N], F32, tag="ph")
            pl = stg.tile([LO, SUB, N], F32, tag="pl")
            rh_t = stg.tile([HI, SUB, N], BF16, tag="rh")
            rl_t = stg.tile([LO, SUB, N], BF16, tag="rl")
            nc.sync.dma_start(out=dh, in_=D[ms:ms + SUB, 0:HI, :].rearrange("m p n -> p m n"))
            nc.sync.dma_start(out=dl, in_=D[ms:ms + SUB, HI:N, :].rearrange("m p n -> p m n"))
            nc.sync.dma_start(out=ph, in_=P[ms:ms + SUB, 0:HI, :].rearrange("m p n -> p m n"))
            nc.sync.dma_start(out=pl, in_=P[ms:ms + SUB, HI:N, :].rearrange("m p n -> p m n"))
            nc.sync.dma_start(out=rh_t, in_=R[ms:ms + SUB, 0:HI, :].rearrange("m p n -> p m n"))
            nc.sync.dma_start(out=rl_t, in_=R[ms:ms + SUB, HI:N, :].rearrange("m p n -> p m n"))
            cur_stage = (dh, dl, ph, pl, rh_t, rl_t)
        dh, dl, ph, pl, rh_t, rl_t = cur_stage
        rhs_h = rh_t[:, i:i + 2, :]    # [128, 2, 192] bf16
        rhs_l = rl_t[:, i:i + 2, :]

        m_h = [mpool.tile([HI, 2 * N], BF16, tag=f"m{a}h{k}", name=f"m{a}h{k}") for a in range(4)]
        m_l = [mpool.tile([LO, 2 * N], BF16, tag=f"m{a}l{k}", name=f"m{a}l{k}") for a in range(4)]

        # dds = D - P  -> m0 (bf16)
        nc.gpsimd.tensor_tensor(out=m_h[0], in0=dh[:, i:i + 2, :],
                                in1=ph[:, i:i + 2, :], op=ALU.subtract)
        nc.gpsimd.tensor_tensor(out=m_l[0], in0=dl[:, i:i + 2, :],
                                in1=pl[:, i:i + 2, :], op=ALU.subtract)

        # chain: 8 products; lhsT alternates m_t, q_t; rhs = R always
        cur_h, cur_l = m_h[0], m_l[0]
        m4_h = m4_l = None
        for t in range(8):
            o_h = ps_ch.tile([HI, 2 * N], F32, tag="po_h")
            o_l = ps_ch.tile([LO, 2 * N], F32, tag="po_l")
            for c in range(2):
                cs = c * N
                lh = cur_h[:, cs:cs + N]
                ll = cur_l[:, cs:cs + N]
                rh = rhs_h[:, c, :]
                rl = rhs_l[:, c, :]
                nc.tensor.matmul(o_h[:, cs:cs + N], lhsT=lh[:, 0:HI], rhs=rh,
                                 start=True, stop=False)
                nc.tensor.matmul(o_h[:, cs:cs + N], lhsT=ll[:, 0:HI], rhs=rl,
                                 start=False, stop=True)
                nc.tensor.matmul(o_l[:, cs:cs + N], lhsT=lh[:, HI:N], rhs=rh,
                                 start=True, stop=False)
                nc.tensor.matmul(o_l[:, cs:cs + N], lhsT=ll[:, HI:N], rhs=rl,
                                 start=False, stop=True)
            # copy PSUM -> SBUF bf16
            if t % 2 == 0:   # q_{t//2+1}
                n_h = qpool.tile([HI, 2 * N], BF16, tag="qh")
                n_l = qpool.tile([LO, 2 * N], BF16, tag="ql")
            elif t == 7:     # m_4: moment-only, lives in the rotating pool
                n_h = qpool.tile([HI, 2 * N], BF16, tag="qh")
                n_l = qpool.tile([LO, 2 * N], BF16, tag="ql")
                m4_h, m4_l = n_h, n_l
            else:            # m_{(t+1)//2}
                a = (t + 1) // 2
                n_h, n_l = m_h[a], m_l[a]
            eh, el = CP_HI[t], CP_LO[t]
            if eh == "act":
                nc.scalar.activation(out=n_h, in_=o_h, func=ACTF.Copy)
            else:
                _eng(nc, eh).tensor_copy(n_h, o_h)
            if el == "act":
                nc.scalar.activation(out=n_l, in_=o_l, func=ACTF.Copy)
            else:
                _eng(nc, el).tensor_copy(n_l, o_l)
            cur_h, cur_l = n_h, n_l

        # moments: 9 elementwise products -> PE colsum (scr is symmetric, so
        # colsum == full inner-product partials).  Matmul out base partition
        # must be 0/32/64, so moments pack 3-per-PSUM-tile at those offsets.
        nmom = len(MOM_PAIRS)
        SM = ps_sm.tile([nmom, 2 * N], F32, tag="SM")
        for t, (a, b) in enumerate(MOM_PAIRS):
            ta_h = m_h[a] if a < 4 else m4_h
            ta_l = m_l[a] if a < 4 else m4_l
            tb_h = m_h[b] if b < 4 else m4_h
            tb_l = m_l[b] if b < 4 else m4_l
            sc_h = scrp.tile([HI, 2 * N], BF16, tag="sc_h")
            sc_l = scrp.tile([LO, 2 * N], BF16, tag="sc_l")
            me = _eng(nc, MOM_ENG[t])
            me.tensor_tensor(out=sc_h, in0=ta_h, in1=tb_h, op=ALU.mult)
            me.tensor_tensor(out=sc_l, in0=ta_l, in1=tb_l, op=ALU.mult)
            # one-hot selector column t: colsum lands in SM row t, +0 rows
            nc.tensor.matmul(SM, lhsT=sel_hi[:, nmom * t:nmom * (t + 1)],
                             rhs=sc_h, start=(t == 0), stop=False)
            nc.tensor.matmul(SM, lhsT=sel_lo[:, nmom * t:nmom * (t + 1)],
                             rhs=sc_l, start=False, stop=(t == nmom - 1))
        # stage2: per-mol moment sums -> SG[0:9, 2k:2k+2]
        nc.vector.tensor_reduce(out=SG[0:nmom, 2 * k:2 * k + 2],
                                in_=SM.rearrange("t (m n) -> t m n", m=2),
                                axis=mybir.AxisListType.X, op=ALU.add)
        pair_ms.append((m_h, m_l))

    # --- solve: transpose -> Hankel build -> batched Gauss ---
    G32 = solvep.tile([GRP, GRP], F32, tag="G32")
    nc.vector.transpose(G32, SG)
    ut = solvep.tile([GRP, 7], F32, tag="ut")
    tmp = solvep.tile([GRP, 7], F32, tag="tmp")
    nc.vector.tensor_add(tmp, G32[:, 2:9], G32[:, 0:7])
    nc.vector.scalar_tensor_tensor(out=ut, in0=G32[:, 1:8], scalar=-2.0,
                                   in1=tmp, op0=ALU.mult, op1=ALU.add)
    ws = solvep.tile([GRP, 14], F32, tag="ws")
    nc.vector.tensor_copy(ws[:, 0:3], ut[:, 0:3])
    nc.vector.tensor_copy(ws[:, 3:6], ut[:, 2:5])
    nc.vector.tensor_copy(ws[:, 6:10], ut[:, 3:7])
    nc.vector.tensor_sub(ws[:, 10:14], G32[:, 1:5], G32[:, 0:4])
    scr = solvep.tile([GRP, 16], F32, tag="scr")
    ys = _solve_sym4(nc, ws, scr)
    beta = solvep.tile([GRP, RANK], F32, tag="beta")
    for a in range(RANK):
        nc.vector.tensor_scalar(out=beta[:, a:a + 1], in0=ys[a], scalar1=-1.0,
                                scalar2=None, op0=ALU.mult)

    # --- combo + DMA out (per 8-mol sub-batch) ---
    for s in range(nsub):
        ms = m0g + s * SUB
        oh = outp.tile([HI, SUB, N], F32, tag="oh")
        ol = outp.tile([LO, SUB, N], F32, tag="ol")
        for j in range(SUB):
            jg = s * SUB + j
            k, c = divmod(jg, 2)
            cs = c * N
            m_h, m_l = pair_ms[k]
            mask = smallp.tile([GRP, RANK], F32, tag="mask")
            nc.vector.tensor_scalar(out=mask, in0=beta,
                                    scalar1=id32[:, jg:jg + 1], scalar2=None,
                                    op0=ALU.mult)
            bc = ps_bc.tile([HI, RANK], F32, tag="bc")
            nc.tensor.matmul(bc, lhsT=ones32, rhs=mask, start=True, stop=True)
            bsb = smallp.tile([HI, RANK], F32, tag="bsb")
            nc.vector.tensor_copy(bsb, bc)

            osh = oh[:, j, :]
            osl = ol[:, j, :]
            for a in range(RANK):
                eh = _eng(nc, CMB_HI[a]) if CMB_HI[a] != "act" else None
                el = _eng(nc, CMB_LO[a]) if CMB_LO[a] != "act" else None
                if a == 0:
                    if CMB_HI[0] == "act":
                        nc.scalar.mul(osh, m_h[0][:, cs:cs + N], bsb[:, 0:1])
                    else:
                        eh.tensor_scalar(out=osh, in0=m_h[0][:, cs:cs + N],
                                         scalar1=bsb[:, 0:1], scalar2=None,
                                         op0=ALU.mult)
                    if CMB_LO[0] == "act":
                        nc.scalar.mul(osl, m_l[0][:, cs:cs + N], bsb[0:LO, 0:1])
                    else:
                        el.tensor_scalar(out=osl, in0=m_l[0][:, cs:cs + N],
                                         scalar1=bsb[0:LO, 0:1], scalar2=None,
                                         op0=ALU.mult)
                else:
                    eh.scalar_tensor_tensor(out=osh, in0=m_h[a][:, cs:cs + N],
                                            scalar=bsb[:, a:a + 1], in1=osh,
                                            op0=ALU.mult, op1=ALU.add)
                    el.scalar_tensor_tensor(out=osl, in0=m_l[a][:, cs:cs + N],
                                            scalar=bsb[0:LO, a:a + 1], in1=osl,
                                            op0=ALU.mult, op1=ALU.add)
        nc.sync.dma_start(out=OUT[ms:ms + SUB, 0:HI, :].rearrange("m p n -> p m n"), in_=oh)
        nc.sync.dma_start(out=OUT[ms:ms + SUB, HI:N, :].rearrange("m p n -> p m n"), in_=ol)


def _solve_sym4(nc, g, s):
    """Batched symmetric 4x4 solve on [GRP,1] column APs.

    g: [GRP, 14] tile, cols 0..9 = O (00,10,11,20,21,22,30,31,32,33),
    cols 10..13 = rhs c.  s: [GRP, 16] scratch.  Returns y col APs.
    """
    def col(t, i):
        return t[:, i:i + 1]

    a, bb, e, c, f, h, d, gg, i_, jj = (col(g, i) for i in range(10))
    r0, r1, r2, r3 = (col(g, 10 + i) for i in range(4))
    p0, p1, p2, p3 = (col(s, 4 + i) for i in range(4))
    l1, l2, l3 = (col(s, 8 + i) for i in range(3))
    t0 = col(s, 11)
    y0, y1, y2, y3 = (col(s, i) for i in range(4))

    mul = nc.vector.tensor_mul
    sub = nc.vector.tensor_sub
    rec = nc.vector.reciprocal

    def upd(x, l, src):  # x -= l*src
        mul(t0, l, src)
        sub(x, x, t0)

    rec(p0, a)
    mul(l1, bb, p0); mul(l2, c, p0); mul(l3, d, p0)
    upd(e, l1, bb); upd(f, l2, bb); upd(gg, l3, bb)
    upd(h, l2, c); upd(i_, l3, c); upd(jj, l3, d)
    upd(r1, l1, r0); upd(r2, l2, r0); upd(r3, l3, r0)

    rec(p1, e)
    mul(l2, f, p1); mul(l3, gg, p1)
    upd(h, l2, f); upd(i_, l3, f); upd(jj, l3, gg)
    upd(r2, l2, r1); upd(r3, l3, r1)

    rec(p2, h)
    mul(l3, i_, p2)
    upd(jj, l3, i_); upd(r3, l3, r2)

    rec(p3, jj)
    mul(y3, r3, p3)
    upd(r2, i_, y3); mul(y2, r2, p2)
    upd(r1, f, y2); upd(r1, gg, y3); mul(y1, r1, p1)
    upd(r0, bb, y1); upd(r0, c, y2); upd(r0, d, y3); mul(y0, r0, p0)
    return [y0, y1, y2, y3]


_NC_CACHE = None


def _get_nc():
    global _NC_CACHE
    if _NC_CACHE is None:
        _NC_CACHE = build_core_kernel()
    return _NC_CACHE


def _to_bf16(x):
    u = np.ascontiguousarray(x, dtype=np.float32).view(np.uint32)
    r = ((u + 0x7FFF + ((u >> 16) & 1)) >> 16).astype(np.uint16)
    return r


def kernel(D, P, R, max_rank=4, _trace=False):
    D = np.ascontiguousarray(D, dtype=np.float32)
    P = np.ascontiguousarray(P, dtype=np.float32)
    Rb = _to_bf16(R)
    nc = _get_nc()
    in_maps = []
    for i in range(NCORES):
        sl = slice(i * MPC, (i + 1) * MPC)
        in_maps.append({"D": D[sl], "P": P[sl], "Rb": Rb[sl]})
    res = run_bass_kernel_spmd(nc, in_maps, core_ids=list(range(NCORES)),
                               trace=_trace)
    out = np.concatenate([r["OUT"] for r in res.results], axis=0)
    if _trace:
        kernel.last_exec_time_ns = res.exec_time_ns
        kernel.last_trace = res.instructions_and_trace
    return out


# revision 5
# speedup vs baseline: 1.0857x; 1.0857x over previous
"""XL-BOMD rank-4 Krylov propagation (EnergyXL) on 8 TRN2 NeuronCores.

Monomial-Krylov reformulation: the reference's Gram-Schmidt basis is
mathematically irrelevant to the output because
    out = -V (W^T W)^{-1} W^T dDS
is invariant under any change of Krylov basis V (W = L V transforms
covariantly).  So per molecule we compute the raw sandwich chain
    m_0 = D - P,  m_{t+1} = R m_t R        (8 PE matmuls, bf16)
its 9 Hankel moments  s_t = <m_a, m_b> (a+b = t)  via the identity
<R a R, b> = <a, R b R>, solve the 4x4 Hankel system
    A[a,c] = s[a+c+2] - 2 s[a+c+1] + s[a+c],  rhs[a] = s[a+1] - s[a]
(batched no-pivot Gaussian elimination over mol-partition layout), and
emit  out = -sum_a y_a m_a.   Rel-err vs fp64 reference: ~2.8e-3
(bf16 matmul chain + bf16 product tiles, fp32 moment accumulation).

Layout: molecules processed in PAIRS sharing [128, 384] (hi rows 0:128)
and [64, 384] (lo rows 128:192) tiles, mol A cols 0:192 / mol B cols
192:384.  Moments: DVE elementwise products (bf16, 2x mode) -> PE
one-hot-selector matmuls (product tiles are symmetric, so the column
sum IS the inner-product partial; selector column t routes it to row t
of a single [9, 384] PSUM bank, other rows accumulate +0) -> one DVE
reduce -> [9, 2] per pair.  Per group of 32 mols: one 32x32 DVE
transpose + one batched Gauss solve + per-mol beta broadcast
(masked ones-matmul).

Engine notes (neuronxcc BIR verifier constraints found the hard way):
GPSIMD(Pool) cannot read PSUM and cannot run TensorScalarPtr (any
per-partition-scalar op, incl. scalar_tensor_tensor); tensor_tensor
may read at most ONE input from PSUM; matmul PSUM out base partition
must be 0/32/64.  Hence: PSUM->SBUF copies on ACT (hi) / DVE (lo),
moment mults + combo FMAs on DVE, dds on Pool.
Moment stages are software-pipelined one pair behind the chain: the PE
queue executes in program order, so a pair's colsum matmuls (gated on
DVE products) must be emitted AFTER the next pair's chain matmuls or
they stall the PE.  TimelineSim estimate: ~484 us/core vs ~3639 us for
the direct GS-on-device baseline (kernel_baseline.py).
"""

import sys

sys.path.insert(0, "/opt/trn_rl_repo")

import numpy as np

import concourse.bass as bass
import concourse.bacc as bacc
import concourse.tile as tile
from concourse import mybir
from concourse.bass_utils import run_bass_kernel_spmd

F32 = mybir.dt.float32
BF16 = mybir.dt.bfloat16
ALU = mybir.AluOpType
ACTF = mybir.ActivationFunctionType

NMOL, N, RANK = 512, 192, 4
NCORES = 8
MPC = NMOL // NCORES      # 64 molecules per core
HI, LO = 128, 64
GRP = 32                  # molecules per solve group
SUB = 8                   # molecules per DMA batch

# moment t -> (a, b) with a+b = t over stored m_0..m_4
MOM_PAIRS = [(0, 0), (0, 1), (1, 1), (1, 2), (2, 2), (2, 3), (3, 3), (3, 4), (4, 4)]

# --- engine assignment tables (tuned against TimelineSim) ---
# product index 0..7 = q1,m1,q2,m2,q3,m3,q4,m4 ; m4 (idx 7) is not copied.
CP_HI = ["act", "act", "act", "act", "act", "act", "act", "act"]
CP_LO = ["dve", "dve", "dve", "dve", "dve", "dve", "dve", "dve"]
MOM_ENG = ["dve"] * 9
CMB_HI = ["act", "dve", "dve", "dve"]   # per a=0..3 (a=0 may be "act")
CMB_LO = ["act", "dve", "dve", "dve"]


def _eng(nc, name):
    return {"dve": nc.vector, "pool": nc.gpsimd}[name]


def build_core_kernel(n_mols=MPC):
    nc = bacc.Bacc(None, target_bir_lowering=False, enable_partition_id=False)
    D = nc.dram_tensor("D", [n_mols, N, N], F32, kind="ExternalInput")
    P = nc.dram_tensor("P", [n_mols, N, N], F32, kind="ExternalInput")
    R = nc.dram_tensor("Rb", [n_mols, N, N], BF16, kind="ExternalInput")
    OUT = nc.dram_tensor("OUT", [n_mols, N, N], F32, kind="ExternalOutput")
    with tile.TileContext(nc) as tc:
        _body(nc, tc, D, P, R, OUT, n_mols)
    nc.finalize()
    return nc


def _body(nc, tc, D, P, R, OUT, n_mols):
    import contextlib

    ctx = contextlib.ExitStack()
    with ctx:
        consts = ctx.enter_context(tc.tile_pool(name="consts", bufs=1))
        stg = ctx.enter_context(tc.tile_pool(name="stg", bufs=2))
        mpool = ctx.enter_context(tc.tile_pool(name="mpool", bufs=1))
        qpool = ctx.enter_context(tc.tile_pool(name="qpool", bufs=6))
        scrp = ctx.enter_context(tc.tile_pool(name="scrp", bufs=2))
        outp = ctx.enter_context(tc.tile_pool(name="outp", bufs=2))
        smallp = ctx.enter_context(tc.tile_pool(name="smallp", bufs=2))
        solvep = ctx.enter_context(tc.tile_pool(name="solvep", bufs=2))
        ps_ch = ctx.enter_context(tc.tile_pool(name="ps_ch", bufs=2, space="PSUM"))
        ps_sm = ctx.enter_context(tc.tile_pool(name="ps_sm", bufs=2, space="PSUM"))
        ps_bc = ctx.enter_context(tc.tile_pool(name="ps_bc", bufs=2, space="PSUM"))

        nmom = len(MOM_PAIRS)
        sel_hi = consts.tile([HI, nmom * nmom], BF16)
        nc.vector.memset(sel_hi, 0.0)
        sel_lo = consts.tile([LO, nmom * nmom], BF16)
        nc.vector.memset(sel_lo, 0.0)
        for t in range(nmom):
            nc.vector.memset(sel_hi[:, nmom * t + t: nmom * t + t + 1], 1.0)
            nc.vector.memset(sel_lo[:, nmom * t + t: nmom * t + t + 1], 1.0)
        ones32 = consts.tile([GRP, HI], F32)
        nc.vector.memset(ones32, 1.0)
        id32 = consts.tile([GRP, GRP], F32)
        idt = consts.tile([GRP, GRP], mybir.dt.int32)
        nc.gpsimd.iota(idt, pattern=[[-1, GRP]], base=0, channel_multiplier=1)
        nc.vector.tensor_scalar(out=id32, in0=idt, scalar1=0, scalar2=None,
                                op0=ALU.is_equal)

        for g in range(n_mols // GRP):
            _group(nc, tc, D, P, R, OUT, g, consts, stg, mpool, qpool, scrp,
                   outp, smallp, solvep, ps_ch, ps_sm, ps_bc,
                   sel_hi, sel_lo, ones32, id32)


def _group(nc, tc, D, P, R, OUT, g, consts, stg, mpool, qpool, scrp, outp,
           smallp, solvep, ps_ch, ps_sm, ps_bc, sel_hi, sel_lo, ones32, id32):
    m0g = g * GRP
    npairs = GRP // 2
    nsub = GRP // SUB

    SG = solvep.tile([GRP, GRP], F32, tag="SG")

    # --- per-pair: staged DMA (rotating per 8-mol sub-batch), dds, chain,
    # moments ---
    pair_ms = []   # per pair: list of 4 (hi, lo) m tiles (bf16, [*, 384])
    pending = []   # pairs whose moment stage is not yet emitted
    cur_stage = None
    for k in range(npairs):
        s, i = divmod(2 * k, SUB)
        if i == 0:
            ms = m0g + s * SUB
            dh = stg.tile([HI, SUB, N], F32, tag="dh")
            dl = stg.tile([LO, SUB, N], F32, tag="dl")
            ph = stg.tile([HI, SUB, N], F32, tag="ph")
            pl = stg.tile([LO, SUB, N], F32, tag="pl")
            rh_t = stg.tile([HI, SUB, N], BF16, tag="rh")
            rl_t = stg.tile([LO, SUB, N], BF16, tag="rl")
            nc.sync.dma_start(out=dh, in_=D[ms:ms + SUB, 0:HI, :].rearrange("m p n -> p m n"))
            nc.sync.dma_start(out=dl, in_=D[ms:ms + SUB, HI:N, :].rearrange("m p n -> p m n"))
            nc.sync.dma_start(out=ph, in_=P[ms:ms + SUB, 0:HI, :].rearrange("m p n -> p m n"))
            nc.sync.dma_start(out=pl, in_=P[ms:ms + SUB, HI:N, :].rearrange("m p n -> p m n"))
            nc.sync.dma_start(out=rh_t, in_=R[ms:ms + SUB, 0:HI, :].rearrange("m p n -> p m n"))
            nc.sync.dma_start(out=rl_t, in_=R[ms:ms + SUB, HI:N, :].rearrange("m p n -> p m n"))
            cur_stage = (dh, dl, ph, pl, rh_t, rl_t)
        dh, dl, ph, pl, rh_t, rl_t = cur_stage
        rhs_h = rh_t[:, i:i + 2, :]    # [128, 2, 192] bf16
        rhs_l = rl_t[:, i:i + 2, :]

        m_h = [mpool.tile([HI, 2 * N], BF16, tag=f"m{a}h{k}", name=f"m{a}h{k}") for a in range(4)]
        m_l = [mpool.tile([LO, 2 * N], BF16, tag=f"m{a}l{k}", name=f"m{a}l{k}") for a in range(4)]

        # dds = D - P  -> m0 (bf16)
        nc.gpsimd.tensor_tensor(out=m_h[0], in0=dh[:, i:i + 2, :],
                                in1=ph[:, i:i + 2, :], op=ALU.subtract)
        nc.gpsimd.tensor_tensor(out=m_l[0], in0=dl[:, i:i + 2, :],
                                in1=pl[:, i:i + 2, :], op=ALU.subtract)

        # chain: 8 products; lhsT alternates m_t, q_t; rhs = R always
        cur_h, cur_l = m_h[0], m_l[0]
        m4_h = m4_l = None
        for t in range(8):
            o_h = ps_ch.tile([HI, 2 * N], F32, tag="po_h")
            o_l = ps_ch.tile([LO, 2 * N], F32, tag="po_l")
            for c in range(2):
                cs = c * N
                lh = cur_h[:, cs:cs + N]
                ll = cur_l[:, cs:cs + N]
                rh = rhs_h[:, c, :]
                rl = rhs_l[:, c, :]
                nc.tensor.matmul(o_h[:, cs:cs + N], lhsT=lh[:, 0:HI], rhs=rh,
                                 start=True, stop=False)
                nc.tensor.matmul(o_h[:, cs:cs + N], lhsT=ll[:, 0:HI], rhs=rl,
                                 start=False, stop=True)
                nc.tensor.matmul(o_l[:, cs:cs + N], lhsT=lh[:, HI:N], rhs=rh,
                                 start=True, stop=False)
                nc.tensor.matmul(o_l[:, cs:cs + N], lhsT=ll[:, HI:N], rhs=rl,
                                 start=False, stop=True)
            # copy PSUM -> SBUF bf16
            if t % 2 == 0:   # q_{t//2+1}
                n_h = qpool.tile([HI, 2 * N], BF16, tag="qh")
                n_l = qpool.tile([LO, 2 * N], BF16, tag="ql")
            elif t == 7:     # m_4: moment-only, lives in the rotating pool
                n_h = qpool.tile([HI, 2 * N], BF16, tag="qh")
                n_l = qpool.tile([LO, 2 * N], BF16, tag="ql")
                m4_h, m4_l = n_h, n_l
            else:            # m_{(t+1)//2}
                a = (t + 1) // 2
                n_h, n_l = m_h[a], m_l[a]
            eh, el = CP_HI[t], CP_LO[t]
            if eh == "act":
                nc.scalar.activation(out=n_h, in_=o_h, func=ACTF.Copy)
            else:
                _eng(nc, eh).tensor_copy(n_h, o_h)
            if el == "act":
                nc.scalar.activation(out=n_l, in_=o_l, func=ACTF.Copy)
            else:
                _eng(nc, el).tensor_copy(n_l, o_l)
            cur_h, cur_l = n_h, n_l

        # moments are deferred one pair (see _pair_moments): the PE queue is
        # in-order, so emitting pair k's colsums before pair k+1's chain
        # matmuls would stall the PE on pair k's DVE multiplies.
        pending.append((k, m_h, m_l, m4_h, m4_l))
        if len(pending) >= 2:
            _pair_moments(nc, pending.pop(0), scrp, ps_sm, SG, sel_hi, sel_lo)
        pair_ms.append((m_h, m_l))

    for item in pending:
        _pair_moments(nc, item, scrp, ps_sm, SG, sel_hi, sel_lo)

    # --- solve: transpose -> Hankel build -> batched Gauss ---
    G32 = solvep.tile([GRP, GRP], F32, tag="G32")
    nc.vector.transpose(G32, SG)
    ut = solvep.tile([GRP, 7], F32, tag="ut")
    tmp = solvep.tile([GRP, 7], F32, tag="tmp")
    nc.vector.tensor_add(tmp, G32[:, 2:9], G32[:, 0:7])
    nc.vector.scalar_tensor_tensor(out=ut, in0=G32[:, 1:8], scalar=-2.0,
                                   in1=tmp, op0=ALU.mult, op1=ALU.add)
    ws = solvep.tile([GRP, 14], F32, tag="ws")
    nc.vector.tensor_copy(ws[:, 0:3], ut[:, 0:3])
    nc.vector.tensor_copy(ws[:, 3:6], ut[:, 2:5])
    nc.vector.tensor_copy(ws[:, 6:10], ut[:, 3:7])
    nc.vector.tensor_sub(ws[:, 10:14], G32[:, 1:5], G32[:, 0:4])
    scr = solvep.tile([GRP, 16], F32, tag="scr")
    ys = _solve_sym4(nc, ws, scr)
    beta = solvep.tile([GRP, RANK], F32, tag="beta")
    for a in range(RANK):
        nc.vector.tensor_scalar(out=beta[:, a:a + 1], in0=ys[a], scalar1=-1.0,
                                scalar2=None, op0=ALU.mult)

    # --- combo + DMA out (per 8-mol sub-batch) ---
    for s in range(nsub):
        ms = m0g + s * SUB
        oh = outp.tile([HI, SUB, N], F32, tag="oh")
        ol = outp.tile([LO, SUB, N], F32, tag="ol")
        for j in range(SUB):
            jg = s * SUB + j
            k, c = divmod(jg, 2)
            cs = c * N
            m_h, m_l = pair_ms[k]
            mask = smallp.tile([GRP, RANK], F32, tag="mask")
            nc.vector.tensor_scalar(out=mask, in0=beta,
                                    scalar1=id32[:, jg:jg + 1], scalar2=None,
                                    op0=ALU.mult)
            bc = ps_bc.tile([HI, RANK], F32, tag="bc")
            nc.tensor.matmul(bc, lhsT=ones32, rhs=mask, start=True, stop=True)
            bsb = smallp.tile([HI, RANK], F32, tag="bsb")
            nc.vector.tensor_copy(bsb, bc)

            osh = oh[:, j, :]
            osl = ol[:, j, :]
            for a in range(RANK):
                eh = _eng(nc, CMB_HI[a]) if CMB_HI[a] != "act" else None
                el = _eng(nc, CMB_LO[a]) if CMB_LO[a] != "act" else None
                if a == 0:
                    if CMB_HI[0] == "act":
                        nc.scalar.mul(osh, m_h[0][:, cs:cs + N], bsb[:, 0:1])
                    else:
                        eh.tensor_scalar(out=osh, in0=m_h[0][:, cs:cs + N],
                                         scalar1=bsb[:, 0:1], scalar2=None,
                                         op0=ALU.mult)
                    if CMB_LO[0] == "act":
                        nc.scalar.mul(osl, m_l[0][:, cs:cs + N], bsb[0:LO, 0:1])
                    else:
                        el.tensor_scalar(out=osl, in0=m_l[0][:, cs:cs + N],
                                         scalar1=bsb[0:LO, 0:1], scalar2=None,
                                         op0=ALU.mult)
                else:
                    eh.scalar_tensor_tensor(out=osh, in0=m_h[a][:, cs:cs + N],
                                            scalar=bsb[:, a:a + 1], in1=osh,
                                            op0=ALU.mult, op1=ALU.add)
                    el.scalar_tensor_tensor(out=osl, in0=m_l[a][:, cs:cs + N],
                                            scalar=bsb[0:LO, a:a + 1], in1=osl,
                                            op0=ALU.mult, op1=ALU.add)
        nc.sync.dma_start(out=OUT[ms:ms + SUB, 0:HI, :].rearrange("m p n -> p m n"), in_=oh)
        nc.sync.dma_start(out=OUT[ms:ms + SUB, HI:N, :].rearrange("m p n -> p m n"), in_=ol)


def _pair_moments(nc, item, scrp, ps_sm, SG, sel_hi, sel_lo):
    """Emit one pair's 9 moment products + selector colsums + stage2."""
    k, m_h, m_l, m4_h, m4_l = item
    nmom = len(MOM_PAIRS)
    SM = ps_sm.tile([nmom, 2 * N], F32, tag="SM")
    for t, (a, b) in enumerate(MOM_PAIRS):
        ta_h = m_h[a] if a < 4 else m4_h
        ta_l = m_l[a] if a < 4 else m4_l
        tb_h = m_h[b] if b < 4 else m4_h
        tb_l = m_l[b] if b < 4 else m4_l
        # symmetry: sum over the lo block (rows 128:192, all cols) equals the
        # sum over the hi right band (rows 0:128, cols 128:192) -- already in
        # the hi product -- plus the 64x64 corner.  So the lo multiply shrinks
        # to the corner only; the PE takes one extra band colsum instead.
        sc_h = scrp.tile([HI, 2 * N], BF16, tag="sc_h")
        sc_c = scrp.tile([LO, 2 * LO], BF16, tag="sc_c")
        me = _eng(nc, MOM_ENG[t])
        me.tensor_tensor(out=sc_h, in0=ta_h, in1=tb_h, op=ALU.mult)
        me.tensor_tensor(
            out=sc_c,
            in0=ta_l.rearrange("p (m n) -> p m n", m=2)[:, :, HI:N],
            in1=tb_l.rearrange("p (m n) -> p m n", m=2)[:, :, HI:N],
            op=ALU.mult)
        # one-hot selector column t: colsums land in SM row t, +0 rows.
        # t=0's full-width write initializes all 9 rows; band/corner colsums
        # accumulate into already-initialized addresses.
        sh = sel_hi[:, nmom * t:nmom * (t + 1)]
        sl = sel_lo[:, nmom * t:nmom * (t + 1)]
        nc.tensor.matmul(SM, lhsT=sh, rhs=sc_h, start=(t == 0), stop=False)
        nc.tensor.matmul(
            SM.rearrange("t (m n) -> t m n", m=2)[:, :, 0:LO], lhsT=sh,
            rhs=sc_h.rearrange("p (m n) -> p m n", m=2)[:, :, HI:N],
            start=False, stop=False)
        nc.tensor.matmul(
            SM.rearrange("t (m n) -> t m n", m=2)[:, :, LO:2 * LO], lhsT=sl,
            rhs=sc_c, start=False, stop=(t == nmom - 1))
    # stage2: per-mol moment sums -> SG[0:9, 2k:2k+2]
    nc.vector.tensor_reduce(out=SG[0:nmom, 2 * k:2 * k + 2],
                            in_=SM.rearrange("t (m n) -> t m n", m=2),
                            axis=mybir.AxisListType.X, op=ALU.add)


def _solve_sym4(nc, g, s):
    """Batched symmetric 4x4 solve on [GRP,1] column APs.

    g: [GRP, 14] tile, cols 0..9 = O (00,10,11,20,21,22,30,31,32,33),
    cols 10..13 = rhs c.  s: [GRP, 16] scratch.  Returns y col APs.
    """
    def col(t, i):
        return t[:, i:i + 1]

    a, bb, e, c, f, h, d, gg, i_, jj = (col(g, i) for i in range(10))
    r0, r1, r2, r3 = (col(g, 10 + i) for i in range(4))
    p0, p1, p2, p3 = (col(s, 4 + i) for i in range(4))
    l1, l2, l3 = (col(s, 8 + i) for i in range(3))
    t0 = col(s, 11)
    y0, y1, y2, y3 = (col(s, i) for i in range(4))

    mul = nc.vector.tensor_mul
    sub = nc.vector.tensor_sub
    rec = nc.vector.reciprocal

    def upd(x, l, src):  # x -= l*src
        mul(t0, l, src)
        sub(x, x, t0)

    rec(p0, a)
    mul(l1, bb, p0); mul(l2, c, p0); mul(l3, d, p0)
    upd(e, l1, bb); upd(f, l2, bb); upd(gg, l3, bb)
    upd(h, l2, c); upd(i_, l3, c); upd(jj, l3, d)
    upd(r1, l1, r0); upd(r2, l2, r0); upd(r3, l3, r0)

    rec(p1, e)
    mul(l2, f, p1); mul(l3, gg, p1)
    upd(h, l2, f); upd(i_, l3, f); upd(jj, l3, gg)
    upd(r2, l2, r1); upd(r3, l3, r1)

    rec(p2, h)
    mul(l3, i_, p2)
    upd(jj, l3, i_); upd(r3, l3, r2)

    rec(p3, jj)
    mul(y3, r3, p3)
    upd(r2, i_, y3); mul(y2, r2, p2)
    upd(r1, f, y2); upd(r1, gg, y3); mul(y1, r1, p1)
    upd(r0, bb, y1); upd(r0, c, y2); upd(r0, d, y3); mul(y0, r0, p0)
    return [y0, y1, y2, y3]


_NC_CACHE = None


def _get_nc():
    global _NC_CACHE
    if _NC_CACHE is None:
        _NC_CACHE = build_core_kernel()
    return _NC_CACHE


def _to_bf16(x):
    u = np.ascontiguousarray(x, dtype=np.float32).view(np.uint32)
    r = ((u + 0x7FFF + ((u >> 16) & 1)) >> 16).astype(np.uint16)
    return r


def kernel(D, P, R, max_rank=4, _trace=False):
    D = np.ascontiguousarray(D, dtype=np.float32)
    P = np.ascontiguousarray(P, dtype=np.float32)
    Rb = _to_bf16(R)
    nc = _get_nc()
    in_maps = []
    for i in range(NCORES):
        sl = slice(i * MPC, (i + 1) * MPC)
        in_maps.append({"D": D[sl], "P": P[sl], "Rb": Rb[sl]})
    res = run_bass_kernel_spmd(nc, in_maps, core_ids=list(range(NCORES)),
                               trace=_trace)
    out = np.concatenate([r["OUT"] for r in res.results], axis=0)
    if _trace:
        kernel.last_exec_time_ns = res.exec_time_ns
        kernel.last_trace = res.instructions_and_trace
    return out
